# revision 31
# baseline (speedup 1.0000x reference)
"""MoE top-1 routing kernel for Trainium2 (8 NeuronCores).

Reference computation (B=8, S=1024, D=768, E=8, F=3072):
    gates = softmax(x @ gate_w + gate_b); expert_idx = argmax(gates)
    out[t] = gelu(x[t] @ w1[e] + b1[e]) @ w2[e] + b2[e]   for e = expert_idx[t]
    (no gate-probability scaling)

Strategy:
  * Routing on host in fp64 (softmax is monotonic, so argmax of logits ==
    argmax of gates).
  * Experts are split into two groups of 4 minimizing |sumA - sumB| of
    token counts.  Cores 0-3 serve group A, cores 4-7 group B; core q of a
    group holds the q-th quarter of the F dimension for its group's
    experts and processes ALL of the group's tokens, producing a partial
    sum of the second matmul that the host reduces.
  * Zero padding via interval refinement: the CT = max(sumA, sumB) token
    stream is cut at the union of both groups' expert boundaries into
    <= 7 "slots".  A slot is a contiguous token range that maps to ONE
    expert per group (a different one for A and B) -- legal because the
    weights are per-core input data.  Only |sumA - sumB| pad tokens are
    wasted, vs ~2.5% for positionwise-max block padding.
  * Matmuls in bf16 with fp32 PSUM accumulation; activations stay
    transposed ([feature, token]).  gelu (erf) on Scalar with the b1 bias
    fused; FFN2 partials copied PSUM->SBUF as bf16 on Vector, DMA'd out.
  * DMA plan (each engine queue is a serialized FIFO descriptor stream;
    rings arbitrate between queues per descriptor, so share ~ descriptor
    size): Sync carries only the token-tile stream (tile-packed flat
    layout => one big contiguous descriptor per partition, 2-buffer
    rolling prefetch), Scalar carries ALL weights strictly in need order
    (slot0 in graduated chunks so the first matmul group lands ASAP),
    GpSimd stays idle through the ramp and carries per-tile output DMAs.
    Slots are processed largest-first, so the run ends on the smallest
    slot and the tail matmul->CAST->DMA chain is short; the last tile's
    output goes out in two halves on Sync so issue+init overlap compute.
    A ~2.3us PE warmup bridges to the first weight chunk and flips the
    HAM clock gate to 2.4 GHz before the real matmul stream starts.
"""

import sys
from itertools import combinations, permutations

try:
    import concourse  # noqa: F401
except ImportError:
    sys.path.insert(0, "/opt/trn_rl_repo")

import numpy as np
import ml_dtypes

import concourse.bass as bass  # noqa: F401
import concourse.tile as tile
import concourse.mybir as mybir
from concourse import bacc
from concourse import bass_utils

BF16 = mybir.dt.bfloat16
F32 = mybir.dt.float32
AF = mybir.ActivationFunctionType

B, S, D, E = 8, 1024, 768, 8
F = 4 * D           # 3072
T = B * S           # 8192
KD = D // 128       # 6 contraction chunks over D
NQ = 4              # F-quarter factor (cores per expert group)
FQ = F // NQ        # 768 features per core
KQ = FQ // 128      # 6 chunks over the F-quarter
N_CORES = 8
MAX_N = 512         # moving-dim tile (one fp32 PSUM bank)

N_WARMUP = 19       # PE warmup matmuls (HAM un-throttle) before real work
LEAD = 128          # width of the very first (ramp) tile

# Debug/profiling knobs (used by the local test harness only).
TRACE = False
LAST_RESULT = None


def _split_tiles(cap, lead=None):
    """Split a block of `cap` tokens into near-equal tiles of <= MAX_N.
    If `lead` is given, the first tile is that size (kept small so the
    very first matmuls depend on only a sliver of the token DMA)."""
    if cap == 0:
        return []
    out = []
    off = 0
    if lead is not None and cap > 2 * lead:
        out.append((0, lead))
        off = lead
        cap -= lead
    n = -(-cap // MAX_N)
    base, rem = divmod(cap, n)
    for i in range(n):
        sz = base + (1 if i < rem else 0)
        out.append((off, sz))
        off += sz
    return out


def plan_slots(counts):
    """Choose the 4/4 expert split and the refined slot structure.

    Returns (groups, comps, caps):
      groups: [listA, listB] expert ids (order = comp part order)
      comps:  per group, list of (expert, span) part widths, sum = CT
      caps:   slot widths in position order (<= 7 slots)
    """
    best = None
    for g0 in combinations(range(E), 4):
        g1 = tuple(e for e in range(E) if e not in g0)
        s0 = sum(counts[e] for e in g0)
        s1 = sum(counts[e] for e in g1)
        if best is None or abs(s0 - s1) < best[0]:
            best = (abs(s0 - s1), list(g0), list(g1), s0, s1)
    _, A, Bg, sA, sB = best
    CT = max(sA, sB)
    padA, padB = CT - sA, CT - sB

    def shift_opts(base_cuts, pad, other_cuts):
        """Candidate monotone shift triples (d1<=d2<=d3<=pad)."""
        cands = [(0, 0, 0), (pad, pad, pad), (0, 0, pad), (0, pad, pad)]
        # try aligning each cut to a nearby other-group cut
        for i in range(3):
            for oc in other_cuts:
                d = oc - base_cuts[i]
                if 0 < d <= pad:
                    tri = [0, 0, 0]
                    for j in range(i, 3):
                        tri[j] = d
                    cands.append(tuple(tri))
        return set(cands)

    bestcfg = None
    for pA in permutations(range(4)):
        cA = [counts[A[i]] for i in pA]
        baseA = [cA[0], cA[0] + cA[1], cA[0] + cA[1] + cA[2]]
        for pB in permutations(range(4)):
            cB = [counts[Bg[i]] for i in pB]
            baseB = [cB[0], cB[0] + cB[1], cB[0] + cB[1] + cB[2]]
            for dA in shift_opts(baseA, padA, baseB):
                cutsA = [baseA[i] + dA[i] for i in range(3)]
                for dB in shift_opts(baseB, padB, cutsA):
                    cutsB = [baseB[i] + dB[i] for i in range(3)]
                    cuts = sorted(set(c for c in cutsA + cutsB
                                      if 0 < c < CT))
                    edges = [0] + cuts + [CT]
                    widths = [edges[i + 1] - edges[i]
                              for i in range(len(edges) - 1)]
                    score = (min(widths), -len(widths))
                    if bestcfg is None or score > bestcfg[0]:
                        partsA = [cA[i] + (dA[i] if i < 3 else padA)
                                  - (dA[i - 1] if i > 0 else 0)
                                  for i in range(4)]
                        # dA are cumulative shifts; part i width =
                        # cA[i] + (d_i - d_{i-1}), last gets pad - d3
                        partsA = [cA[0] + dA[0],
                                  cA[1] + dA[1] - dA[0],
                                  cA[2] + dA[2] - dA[1],
                                  cA[3] + padA - dA[2]]
                        partsB = [cB[0] + dB[0],
                                  cB[1] + dB[1] - dB[0],
                                  cB[2] + dB[2] - dB[1],
                                  cB[3] + padB - dB[2]]
                        bestcfg = (score,
                                   [A[i] for i in pA], partsA,
                                   [Bg[i] for i in pB], partsB,
                                   widths)
    _, ordA, partsA, ordB, partsB, caps = bestcfg
    groups = [ordA, ordB]
    comps = [list(zip(ordA, partsA)), list(zip(ordB, partsB))]
    return groups, comps, caps, CT


def build_program(caps):
    """Per-core program: NS slots with token capacities `caps`."""
    caps = list(caps)
    NS = len(caps)
    CT = sum(caps)
    nc = bacc.Bacc("TRN2", target_bir_lowering=False, debug=False,
                   num_devices=N_CORES)

    # Token/output tensors use a tile-packed flat layout: each schedule
    # tile's [KD, w] block is contiguous per partition, so its DMA is one
    # large contiguous descriptor per partition (the rings arbitrate
    # between queues per descriptor, so big descriptors = big bandwidth
    # share).
    xT_d = nc.dram_tensor("xT", (128, KD * CT), BF16, kind="ExternalInput")
    w1_d = nc.dram_tensor("w1", (128, NS, KQ, KD, 128), BF16,
                          kind="ExternalInput")
    w2_d = nc.dram_tensor("w2", (128, NS, KD, KQ, 128), BF16,
                          kind="ExternalInput")
    b1_d = nc.dram_tensor("b1", (128, NS, KQ), F32, kind="ExternalInput")
    yT_d = nc.dram_tensor("yT", (128, KD * CT), BF16, kind="ExternalOutput")

    offs = np.concatenate([[0], np.cumsum(caps)]).astype(int)
    # Process slots largest-first: good compute-per-weight-load during the
    # ramp, and the run ends on the smallest slot (short tail chain).
    order = sorted(range(NS), key=lambda j: -caps[j])
    sched = []  # (slot, tile-offset-in-CT, width) in execution order
    for idx, j in enumerate(order):
        lead = LEAD if idx == 0 else None
        for (o, w) in _split_tiles(caps[j], lead=lead):
            sched.append((j, offs[j] + o, w))
    # flat offset of each tile in the packed xT/yT layout
    foffs = np.concatenate([[0], np.cumsum([KD * w for (_, _, w)
                                            in sched])]).astype(int)

    with tile.TileContext(nc) as tc:
        with (
            tc.tile_pool(name="wts", bufs=1) as wts,
            tc.tile_pool(name="xio", bufs=2) as xio,
            tc.tile_pool(name="act", bufs=2) as actp,
            tc.tile_pool(name="ps1", bufs=4, space="PSUM") as ps1,
            tc.tile_pool(name="ps2", bufs=4, space="PSUM") as ps2,
        ):
            w1 = wts.tile([128, NS, KQ, KD, 128], BF16, tag="w1")
            w2 = wts.tile([128, NS, KD, KQ, 128], BF16, tag="w2")
            b1 = wts.tile([128, NS, KQ], F32, tag="b1")
            warm = wts.tile([128, 128], BF16, tag="warm")
            nc.vector.memset(warm[:], 0.0)
            wps = ps1.tile([128, 128], F32, tag="ps1",
                           padded_shape=[128, MAX_N])

            # PE warmup: dummy matmuls run while the head DMAs stream in,
            # flipping the HAM clock gate to 2.4 GHz before the real
            # matmul stream starts.
            for _ in range(N_WARMUP):
                nc.tensor.matmul(wps[:, :], warm[:, :], warm[:, :])

            # Each engine queue is a serialized FIFO descriptor stream;
            # the rings arbitrate between queues per DESCRIPTOR, so a
            # queue's bandwidth share scales with its descriptor size and
            # queue order must match need order.  ALL weights go on
            # Scalar, strictly need-ordered (slot0 in graduated chunks so
            # the first matmul group lands ASAP, then whole-slot
            # transfers whose 9KB descriptors win a large share).  Sync
            # carries only token tiles; GpSimd (whose ~930ns software
            # issue cost would also hurt) stays idle during the ramp and
            # carries only output DMAs.
            s0 = order[0]
            nc.scalar.dma_start(w1[:, s0, 0, :3], w1_d[:, s0, 0, :3])
            nc.scalar.dma_start(w1[:, s0, 0, 3:], w1_d[:, s0, 0, 3:])
            nc.scalar.dma_start(b1[:], b1_d[:])
            nc.scalar.dma_start(w1[:, s0, 1:3], w1_d[:, s0, 1:3])
            nc.scalar.dma_start(w1[:, s0, 3:], w1_d[:, s0, 3:])
            nc.scalar.dma_start(w2[:, s0, 0:3], w2_d[:, s0, 0:3])
            nc.scalar.dma_start(w2[:, s0, 3:], w2_d[:, s0, 3:])

            def load_slot(j, halves=False):
                if halves:
                    # halve the descriptors to ring-share fairly with the
                    # concurrent ramp token tiles
                    nc.scalar.dma_start(w1[:, j, :3], w1_d[:, j, :3])
                    nc.scalar.dma_start(w1[:, j, 3:], w1_d[:, j, 3:])
                    nc.scalar.dma_start(w2[:, j, :3], w2_d[:, j, :3])
                    nc.scalar.dma_start(w2[:, j, 3:], w2_d[:, j, 3:])
                else:
                    nc.scalar.dma_start(w1[:, j], w1_d[:, j])
                    nc.scalar.dma_start(w2[:, j], w2_d[:, j])

            def ffn1(j, xt, nt):
                h = actp.tile([128, KQ, nt], BF16, tag="h",
                              padded_shape=[128, KQ, MAX_N])
                for m in range(KQ):
                    ps = ps1.tile([128, nt], F32, tag="ps1",
                                  padded_shape=[128, MAX_N])
                    for k in range(KD):
                        nc.tensor.matmul(
                            ps[:, :],
                            w1[:, j, m, k, :],
                            xt[:, k * nt:(k + 1) * nt],
                            start=(k == 0),
                            stop=(k == KD - 1),
                        )
                    nc.scalar.activation(h[:, m, :], ps[:, :], AF.Gelu,
                                         bias=b1[:, j, m:m + 1])
                return h

            def ffn2(i, j, nt, h, split_out=False):
                fo = int(foffs[i])
                y = actp.tile([128, KD * nt], BF16, tag="y",
                              padded_shape=[128, KD * MAX_N])
                for md in range(KD):
                    ps = ps2.tile([128, nt], F32, tag="ps2",
                                  padded_shape=[128, MAX_N])
                    for k in range(KQ):
                        nc.tensor.matmul(
                            ps[:, :],
                            w2[:, j, md, k, :],
                            h[:, k, :],
                            start=(k == 0),
                            stop=(k == KQ - 1),
                        )
                    nc.vector.tensor_copy(y[:, md * nt:(md + 1) * nt],
                                          ps[:, :])
                    if split_out and md == 2:
                        # last tile, first half: issue early on Sync so
                        # its issue+init overlaps the second half's MMs
                        nc.sync.dma_start(yT_d[:, fo:fo + 3 * nt],
                                          y[:, :3 * nt])
                if split_out:
                    nc.sync.dma_start(yT_d[:, fo + 3 * nt:fo + KD * nt],
                                      y[:, 3 * nt:])
                else:
                    nc.gpsimd.dma_start(yT_d[:, fo:fo + KD * nt], y[:, :])

            def load_xt(i):
                jj, _, w = sched[i]
                fo = int(foffs[i])
                xt = xio.tile([128, KD * w], BF16,
                              tag="xT", padded_shape=[128, KD * MAX_N])
                nc.sync.dma_start(xt[:, :], xT_d[:, fo:fo + KD * w])
                return xt

            # Software-pipelined emission: FFN1(t) ahead of FFN2(t-1) so
            # the PE never waits on the gelu of the tile it just produced.
            xts = {}
            xnext = min(2, len(sched))
            for i in range(xnext):
                xts[i] = load_xt(i)
            slot_pos = {j: p for p, j in enumerate(order)}
            seen = set()
            prev = None
            for i, (j, n0, nt) in enumerate(sched):
                if j not in seen:
                    seen.add(j)
                    p = slot_pos[j]
                    # staggered weight prefetch: slot1 right away (in
                    # halves, ring-sharing fairly with the ramp tiles);
                    # later slots deferred so their 9KB descriptors don't
                    # starve the ramp; extra lookahead for the small tail
                    # slots whose compute is short
                    here = {0: [1], 1: [2, 3], 2: [4], 3: [5, 6]}.get(p, [])
                    for pn in here:
                        if pn < NS:
                            load_slot(order[pn], halves=(pn == 1))
                while xnext < min(i + 3, len(sched)):
                    xts[xnext] = load_xt(xnext)
                    xnext += 1
                h = ffn1(j, xts[i], nt)
                if prev is not None:
                    ffn2(*prev)
                prev = (i, j, nt, h)
            if prev is not None:
                # Last tile: emit the output DMA per-chunk so the
                # transfers hide under the final matmuls.
                ffn2(*prev, split_out=True)

    nc.compile()
    return nc


_PROGRAM_CACHE = {}


def _get_program(caps):
    key = tuple(caps)
    if key not in _PROGRAM_CACHE:
        _PROGRAM_CACHE[key] = build_program(caps)
    return _PROGRAM_CACHE[key]


def kernel(x, gate_w, gate_b, w1, b1, w2, b2):
    x = np.asarray(x)
    w1 = np.asarray(w1)
    b1 = np.asarray(b1)
    w2 = np.asarray(w2)
    b2 = np.asarray(b2)
    xt = x.reshape(T, D)

    # --- Routing on host (fp64; softmax is monotonic => argmax of logits) ---
    logits = xt.astype(np.float64) @ np.asarray(gate_w, np.float64)
    logits += np.asarray(gate_b, np.float64)
    eidx = np.argmax(logits, axis=-1)
    counts = np.bincount(eidx, minlength=E)

    groups, comps, caps, CT = plan_slots(counts)
    NS = len(caps)
    offs = np.concatenate([[0], np.cumsum(caps)]).astype(int)

    # slot -> expert map per group (slot range lies inside one comp part)
    slot_expert = []
    for g in range(2):
        edges = np.concatenate([[0], np.cumsum([p for _, p in comps[g]])])
        se = []
        for jj in range(NS):
            p = np.searchsorted(edges, offs[jj], side="right") - 1
            se.append(comps[g][p][0])
        slot_expert.append(se)

    nc = _get_program(caps)

    # reproduce the program's tile schedule for the packed x/y layouts
    order = sorted(range(NS), key=lambda j: -caps[j])
    sched = []
    for idx, j in enumerate(order):
        lead = LEAD if idx == 0 else None
        for (o, w) in _split_tiles(int(caps[j]), lead=lead):
            sched.append((j, int(offs[j]) + o, w))

    xt_bf = xt.astype(ml_dtypes.bfloat16)
    tok_idx = []      # per group: list of (expert, token_indices, span_start)
    in_maps = [None] * N_CORES
    for g in range(2):
        xg = np.zeros((CT, D), ml_dtypes.bfloat16)
        placement = []
        pos = 0
        for e, span in comps[g]:
            idx = np.nonzero(eidx == e)[0]
            xg[pos:pos + len(idx)] = xt_bf[idx]
            placement.append((e, idx, pos))
            pos += span
        tok_idx.append(placement)
        # [CT, D] -> tile-packed [128, KD*CT] in schedule order
        x3 = xg.T.reshape(KD, 128, CT)
        xTg = np.concatenate(
            [x3[:, :, n0:n0 + w].transpose(1, 0, 2).reshape(128, KD * w)
             for (_, n0, w) in sched], axis=1)
        xTg = np.ascontiguousarray(xTg)
        for q in range(NQ):
            w1q = np.empty((128, NS, KQ, KD, 128), ml_dtypes.bfloat16)
            w2q = np.empty((128, NS, KD, KQ, 128), ml_dtypes.bfloat16)
            b1q = np.empty((128, NS, KQ), np.float32)
            for jj in range(NS):
                e = slot_expert[g][jj]
                w1e = w1[e][:, q * FQ:(q + 1) * FQ]        # [D, FQ]
                w1q[:, jj] = w1e.reshape(KD, 128, KQ, 128).transpose(
                    1, 2, 0, 3).astype(ml_dtypes.bfloat16)
                w2e = w2[e][q * FQ:(q + 1) * FQ, :]        # [FQ, D]
                w2q[:, jj] = w2e.reshape(KQ, 128, KD, 128).transpose(
                    1, 2, 0, 3).astype(ml_dtypes.bfloat16)
                b1q[:, jj] = b1[e][q * FQ:(q + 1) * FQ].reshape(KQ, 128).T
            in_maps[g * NQ + q] = {"xT": xTg, "w1": w1q, "w2": w2q,
                                   "b1": b1q}

    res = bass_utils.run_bass_kernel_spmd(nc, in_maps,
                                          core_ids=list(range(N_CORES)),
                                          trace=TRACE)
    global LAST_RESULT
    LAST_RESULT = res

    out = np.empty((T, D), np.float32)
    for g in range(2):
        acc = res.results[g * NQ]["yT"].astype(np.float32)
        for q in range(1, NQ):
            acc += res.results[g * NQ + q]["yT"].astype(np.float32)
        # tile-packed [128, KD*CT] -> [D, CT] -> [CT, D]
        y3 = np.empty((KD, 128, CT), np.float32)
        fo = 0
        for (_, n0, w) in sched:
            y3[:, :, n0:n0 + w] = acc[:, fo:fo + KD * w].reshape(
                128, KD, w).transpose(1, 0, 2)
            fo += KD * w
        yg = y3.reshape(D, CT).T
        for e, idx, pos in tok_idx[g]:
            out[idx] = yg[pos:pos + len(idx)] + b2[e]
    return out.reshape(B, S, D)


# revision 32
# speedup vs baseline: 1.0222x; 1.0222x over previous
"""MoE top-1 routing kernel for Trainium2 (8 NeuronCores).

Reference computation (B=8, S=1024, D=768, E=8, F=3072):
    gates = softmax(x @ gate_w + gate_b); expert_idx = argmax(gates)
    out[t] = gelu(x[t] @ w1[e] + b1[e]) @ w2[e] + b2[e]   for e = expert_idx[t]
    (no gate-probability scaling)

Strategy:
  * Routing on host in fp64 (softmax is monotonic, so argmax of logits ==
    argmax of gates).
  * Experts are split into two groups of 4 minimizing |sumA - sumB| of
    token counts.  Cores 0-3 serve group A, cores 4-7 group B; core q of a
    group holds the q-th quarter of the F dimension for its group's
    experts and processes ALL of the group's tokens, producing a partial
    sum of the second matmul that the host reduces.
  * Zero padding via interval refinement: the CT = max(sumA, sumB) token
    stream is cut at the union of both groups' expert boundaries into
    <= 7 "slots".  A slot is a contiguous token range that maps to ONE
    expert per group (a different one for A and B) -- legal because the
    weights are per-core input data.  Only |sumA - sumB| pad tokens are
    wasted, vs ~2.5% for positionwise-max block padding.
  * Matmuls in bf16 with fp32 PSUM accumulation; activations stay
    transposed ([feature, token]).  gelu (erf) on Scalar with the b1 bias
    fused; FFN2 partials copied PSUM->SBUF as bf16 on Vector, DMA'd out.
  * DMA plan (each engine queue is a serialized FIFO descriptor stream;
    rings arbitrate between queues per descriptor, so share ~ descriptor
    size): Sync carries only the token-tile stream (tile-packed flat
    layout => one big contiguous descriptor per partition, 2-buffer
    rolling prefetch), Scalar carries ALL weights strictly in need order
    (slot0 in graduated chunks so the first matmul group lands ASAP),
    GpSimd stays idle through the ramp and carries per-tile output DMAs.
    Slots are processed largest-first, so the run ends on the smallest
    slot and the tail matmul->CAST->DMA chain is short; the last tile's
    output goes out in two halves on Sync so issue+init overlap compute.
    A ~2.3us PE warmup bridges to the first weight chunk and flips the
    HAM clock gate to 2.4 GHz before the real matmul stream starts.
"""

import sys
from itertools import combinations, permutations

try:
    import concourse  # noqa: F401
except ImportError:
    sys.path.insert(0, "/opt/trn_rl_repo")

import numpy as np
import ml_dtypes

import concourse.bass as bass  # noqa: F401
import concourse.tile as tile
import concourse.mybir as mybir
from concourse import bacc
from concourse import bass_utils

BF16 = mybir.dt.bfloat16
F32 = mybir.dt.float32
AF = mybir.ActivationFunctionType

B, S, D, E = 8, 1024, 768, 8
F = 4 * D           # 3072
T = B * S           # 8192
KD = D // 128       # 6 contraction chunks over D
NQ = 4              # F-quarter factor (cores per expert group)
FQ = F // NQ        # 768 features per core
KQ = FQ // 128      # 6 chunks over the F-quarter
N_CORES = 8
MAX_N = 512         # moving-dim tile (one fp32 PSUM bank)

N_WARMUP = 22       # PE warmup matmuls (HAM un-throttle) before real work
LEAD = 128          # width of the very first (ramp) tile

# Debug/profiling knobs (used by the local test harness only).
TRACE = False
LAST_RESULT = None


def _split_tiles(cap, lead=None):
    """Split a block of `cap` tokens into near-equal tiles of <= MAX_N.
    If `lead` is given, the first tile is that size (kept small so the
    very first matmuls depend on only a sliver of the token DMA)."""
    if cap == 0:
        return []
    out = []
    off = 0
    if lead is not None and cap > 2 * lead:
        out.append((0, lead))
        off = lead
        cap -= lead
    n = -(-cap // MAX_N)
    base, rem = divmod(cap, n)
    for i in range(n):
        sz = base + (1 if i < rem else 0)
        out.append((off, sz))
        off += sz
    return out


def plan_slots(counts):
    """Choose the 4/4 expert split and the refined slot structure.

    Returns (groups, comps, caps):
      groups: [listA, listB] expert ids (order = comp part order)
      comps:  per group, list of (expert, span) part widths, sum = CT
      caps:   slot widths in position order (<= 7 slots)
    """
    best = None
    for g0 in combinations(range(E), 4):
        g1 = tuple(e for e in range(E) if e not in g0)
        s0 = sum(counts[e] for e in g0)
        s1 = sum(counts[e] for e in g1)
        if best is None or abs(s0 - s1) < best[0]:
            best = (abs(s0 - s1), list(g0), list(g1), s0, s1)
    _, A, Bg, sA, sB = best
    CT = max(sA, sB)
    padA, padB = CT - sA, CT - sB

    def shift_opts(base_cuts, pad, other_cuts):
        """Candidate monotone shift triples (d1<=d2<=d3<=pad)."""
        cands = [(0, 0, 0), (pad, pad, pad), (0, 0, pad), (0, pad, pad)]
        # try aligning each cut to a nearby other-group cut
        for i in range(3):
            for oc in other_cuts:
                d = oc - base_cuts[i]
                if 0 < d <= pad:
                    tri = [0, 0, 0]
                    for j in range(i, 3):
                        tri[j] = d
                    cands.append(tuple(tri))
        return set(cands)

    bestcfg = None
    for pA in permutations(range(4)):
        cA = [counts[A[i]] for i in pA]
        baseA = [cA[0], cA[0] + cA[1], cA[0] + cA[1] + cA[2]]
        for pB in permutations(range(4)):
            cB = [counts[Bg[i]] for i in pB]
            baseB = [cB[0], cB[0] + cB[1], cB[0] + cB[1] + cB[2]]
            for dA in shift_opts(baseA, padA, baseB):
                cutsA = [baseA[i] + dA[i] for i in range(3)]
                for dB in shift_opts(baseB, padB, cutsA):
                    cutsB = [baseB[i] + dB[i] for i in range(3)]
                    cuts = sorted(set(c for c in cutsA + cutsB
                                      if 0 < c < CT))
                    edges = [0] + cuts + [CT]
                    widths = [edges[i + 1] - edges[i]
                              for i in range(len(edges) - 1)]
                    score = (min(widths), -len(widths))
                    if bestcfg is None or score > bestcfg[0]:
                        partsA = [cA[i] + (dA[i] if i < 3 else padA)
                                  - (dA[i - 1] if i > 0 else 0)
                                  for i in range(4)]
                        # dA are cumulative shifts; part i width =
                        # cA[i] + (d_i - d_{i-1}), last gets pad - d3
                        partsA = [cA[0] + dA[0],
                                  cA[1] + dA[1] - dA[0],
                                  cA[2] + dA[2] - dA[1],
                                  cA[3] + padA - dA[2]]
                        partsB = [cB[0] + dB[0],
                                  cB[1] + dB[1] - dB[0],
                                  cB[2] + dB[2] - dB[1],
                                  cB[3] + padB - dB[2]]
                        bestcfg = (score,
                                   [A[i] for i in pA], partsA,
                                   [Bg[i] for i in pB], partsB,
                                   widths)
    _, ordA, partsA, ordB, partsB, caps = bestcfg
    groups = [ordA, ordB]
    comps = [list(zip(ordA, partsA)), list(zip(ordB, partsB))]
    return groups, comps, caps, CT


def build_program(caps):
    """Per-core program: NS slots with token capacities `caps`."""
    caps = list(caps)
    NS = len(caps)
    CT = sum(caps)
    nc = bacc.Bacc("TRN2", target_bir_lowering=False, debug=False,
                   num_devices=N_CORES)

    # Token/output tensors use a tile-packed flat layout: each schedule
    # tile's [KD, w] block is contiguous per partition, so its DMA is one
    # large contiguous descriptor per partition (the rings arbitrate
    # between queues per descriptor, so big descriptors = big bandwidth
    # share).
    xT_d = nc.dram_tensor("xT", (128, KD * CT), BF16, kind="ExternalInput")
    w1_d = nc.dram_tensor("w1", (128, NS, KQ, KD, 128), BF16,
                          kind="ExternalInput")
    w2_d = nc.dram_tensor("w2", (128, NS, KD, KQ, 128), BF16,
                          kind="ExternalInput")
    b1_d = nc.dram_tensor("b1", (128, NS, KQ), F32, kind="ExternalInput")
    yT_d = nc.dram_tensor("yT", (128, KD * CT), BF16, kind="ExternalOutput")

    offs = np.concatenate([[0], np.cumsum(caps)]).astype(int)
    # Process slots largest-first: good compute-per-weight-load during the
    # ramp, and the run ends on the smallest slot (short tail chain).
    order = sorted(range(NS), key=lambda j: -caps[j])
    sched = []  # (slot, tile-offset-in-CT, width) in execution order
    for idx, j in enumerate(order):
        lead = LEAD if idx == 0 else None
        for (o, w) in _split_tiles(caps[j], lead=lead):
            sched.append((j, offs[j] + o, w))
    # flat offset of each tile in the packed xT/yT layout
    foffs = np.concatenate([[0], np.cumsum([KD * w for (_, _, w)
                                            in sched])]).astype(int)

    with tile.TileContext(nc) as tc:
        with (
            tc.tile_pool(name="wts", bufs=1) as wts,
            tc.tile_pool(name="xio", bufs=2) as xio,
            tc.tile_pool(name="act", bufs=2) as actp,
            tc.tile_pool(name="ps1", bufs=4, space="PSUM") as ps1,
            tc.tile_pool(name="ps2", bufs=4, space="PSUM") as ps2,
        ):
            w1 = wts.tile([128, NS, KQ, KD, 128], BF16, tag="w1")
            w2 = wts.tile([128, NS, KD, KQ, 128], BF16, tag="w2")
            b1 = wts.tile([128, NS, KQ], F32, tag="b1")
            warm = wts.tile([128, 128], BF16, tag="warm")
            nc.vector.memset(warm[:], 0.0)
            wps = ps1.tile([128, 128], F32, tag="ps1",
                           padded_shape=[128, MAX_N])

            # PE warmup: dummy matmuls run while the head DMAs stream in,
            # flipping the HAM clock gate to 2.4 GHz before the real
            # matmul stream starts.
            for _ in range(N_WARMUP):
                nc.tensor.matmul(wps[:, :], warm[:, :], warm[:, :])

            # Each engine queue is a serialized FIFO descriptor stream;
            # the rings arbitrate between queues per DESCRIPTOR, so a
            # queue's bandwidth share scales with its descriptor size and
            # queue order must match need order.  ALL weights go on
            # Scalar, strictly need-ordered (slot0 in graduated chunks so
            # the first matmul group lands ASAP, then whole-slot
            # transfers whose 9KB descriptors win a large share).  Sync
            # carries only token tiles; GpSimd (whose ~930ns software
            # issue cost would also hurt) stays idle during the ramp and
            # carries only output DMAs.
            s0 = order[0]
            nc.scalar.dma_start(w1[:, s0, 0], w1_d[:, s0, 0])
            nc.scalar.dma_start(b1[:], b1_d[:])
            nc.scalar.dma_start(w1[:, s0, 1:3], w1_d[:, s0, 1:3])
            nc.scalar.dma_start(w1[:, s0, 3:], w1_d[:, s0, 3:])
            nc.scalar.dma_start(w2[:, s0, 0:3], w2_d[:, s0, 0:3])
            nc.scalar.dma_start(w2[:, s0, 3:], w2_d[:, s0, 3:])

            def load_slot(j, halves=False):
                if halves:
                    # halve the descriptors to ring-share fairly with the
                    # concurrent ramp token tiles
                    nc.scalar.dma_start(w1[:, j, :3], w1_d[:, j, :3])
                    nc.scalar.dma_start(w1[:, j, 3:], w1_d[:, j, 3:])
                    nc.scalar.dma_start(w2[:, j, :3], w2_d[:, j, :3])
                    nc.scalar.dma_start(w2[:, j, 3:], w2_d[:, j, 3:])
                else:
                    nc.scalar.dma_start(w1[:, j], w1_d[:, j])
                    nc.scalar.dma_start(w2[:, j], w2_d[:, j])

            def ffn1(j, xt, nt):
                h = actp.tile([128, KQ, nt], BF16, tag="h",
                              padded_shape=[128, KQ, MAX_N])
                for m in range(KQ):
                    ps = ps1.tile([128, nt], F32, tag="ps1",
                                  padded_shape=[128, MAX_N])
                    for k in range(KD):
                        nc.tensor.matmul(
                            ps[:, :],
                            w1[:, j, m, k, :],
                            xt[:, k * nt:(k + 1) * nt],
                            start=(k == 0),
                            stop=(k == KD - 1),
                        )
                    nc.scalar.activation(h[:, m, :], ps[:, :], AF.Gelu,
                                         bias=b1[:, j, m:m + 1])
                return h

            def ffn2(i, j, nt, h, split_out=False):
                fo = int(foffs[i])
                y = actp.tile([128, KD * nt], BF16, tag="y",
                              padded_shape=[128, KD * MAX_N])
                for md in range(KD):
                    ps = ps2.tile([128, nt], F32, tag="ps2",
                                  padded_shape=[128, MAX_N])
                    for k in range(KQ):
                        nc.tensor.matmul(
                            ps[:, :],
                            w2[:, j, md, k, :],
                            h[:, k, :],
                            start=(k == 0),
                            stop=(k == KQ - 1),
                        )
                    nc.vector.tensor_copy(y[:, md * nt:(md + 1) * nt],
                                          ps[:, :])
                    if split_out and md == 2:
                        # last tile, first half: issue early on Sync so
                        # its issue+init overlaps the second half's MMs
                        nc.sync.dma_start(yT_d[:, fo:fo + 3 * nt],
                                          y[:, :3 * nt])
                if split_out:
                    nc.sync.dma_start(yT_d[:, fo + 3 * nt:fo + KD * nt],
                                      y[:, 3 * nt:])
                else:
                    nc.gpsimd.dma_start(yT_d[:, fo:fo + KD * nt], y[:, :])

            def load_xt(i):
                jj, _, w = sched[i]
                fo = int(foffs[i])
                xt = xio.tile([128, KD * w], BF16,
                              tag="xT", padded_shape=[128, KD * MAX_N])
                nc.sync.dma_start(xt[:, :], xT_d[:, fo:fo + KD * w])
                return xt

            # Software-pipelined emission: FFN1(t) ahead of FFN2(t-1) so
            # the PE never waits on the gelu of the tile it just produced.
            xts = {}
            xnext = min(2, len(sched))
            for i in range(xnext):
                xts[i] = load_xt(i)
            slot_pos = {j: p for p, j in enumerate(order)}
            seen = set()
            prev = None
            for i, (j, n0, nt) in enumerate(sched):
                if j not in seen:
                    seen.add(j)
                    p = slot_pos[j]
                    # staggered weight prefetch: slot1 right away (in
                    # halves, ring-sharing fairly with the ramp tiles);
                    # later slots deferred so their 9KB descriptors don't
                    # starve the ramp; extra lookahead for the small tail
                    # slots whose compute is short
                    here = {0: [1], 1: [2, 3], 2: [4], 3: [5, 6]}.get(p, [])
                    for pn in here:
                        if pn < NS:
                            load_slot(order[pn], halves=(pn == 1))
                while xnext < min(i + 3, len(sched)):
                    xts[xnext] = load_xt(xnext)
                    xnext += 1
                h = ffn1(j, xts[i], nt)
                if prev is not None:
                    ffn2(*prev)
                prev = (i, j, nt, h)
            if prev is not None:
                # Last tile: emit the output DMA per-chunk so the
                # transfers hide under the final matmuls.
                ffn2(*prev, split_out=True)

    nc.compile()
    return nc


_PROGRAM_CACHE = {}


def _get_program(caps):
    key = tuple(caps)
    if key not in _PROGRAM_CACHE:
        _PROGRAM_CACHE[key] = build_program(caps)
    return _PROGRAM_CACHE[key]


def kernel(x, gate_w, gate_b, w1, b1, w2, b2):
    x = np.asarray(x)
    w1 = np.asarray(w1)
    b1 = np.asarray(b1)
    w2 = np.asarray(w2)
    b2 = np.asarray(b2)
    xt = x.reshape(T, D)

    # --- Routing on host (fp64; softmax is monotonic => argmax of logits) ---
    logits = xt.astype(np.float64) @ np.asarray(gate_w, np.float64)
    logits += np.asarray(gate_b, np.float64)
    eidx = np.argmax(logits, axis=-1)
    counts = np.bincount(eidx, minlength=E)

    groups, comps, caps, CT = plan_slots(counts)
    NS = len(caps)
    offs = np.concatenate([[0], np.cumsum(caps)]).astype(int)

    # slot -> expert map per group (slot range lies inside one comp part)
    slot_expert = []
    for g in range(2):
        edges = np.concatenate([[0], np.cumsum([p for _, p in comps[g]])])
        se = []
        for jj in range(NS):
            p = np.searchsorted(edges, offs[jj], side="right") - 1
            se.append(comps[g][p][0])
        slot_expert.append(se)

    nc = _get_program(caps)

    # reproduce the program's tile schedule for the packed x/y layouts
    order = sorted(range(NS), key=lambda j: -caps[j])
    sched = []
    for idx, j in enumerate(order):
        lead = LEAD if idx == 0 else None
        for (o, w) in _split_tiles(int(caps[j]), lead=lead):
            sched.append((j, int(offs[j]) + o, w))

    xt_bf = xt.astype(ml_dtypes.bfloat16)
    tok_idx = []      # per group: list of (expert, token_indices, span_start)
    in_maps = [None] * N_CORES
    for g in range(2):
        xg = np.zeros((CT, D), ml_dtypes.bfloat16)
        placement = []
        pos = 0
        for e, span in comps[g]:
            idx = np.nonzero(eidx == e)[0]
            xg[pos:pos + len(idx)] = xt_bf[idx]
            placement.append((e, idx, pos))
            pos += span
        tok_idx.append(placement)
        # [CT, D] -> tile-packed [128, KD*CT] in schedule order
        x3 = xg.T.reshape(KD, 128, CT)
        xTg = np.concatenate(
            [x3[:, :, n0:n0 + w].transpose(1, 0, 2).reshape(128, KD * w)
             for (_, n0, w) in sched], axis=1)
        xTg = np.ascontiguousarray(xTg)
        for q in range(NQ):
            w1q = np.empty((128, NS, KQ, KD, 128), ml_dtypes.bfloat16)
            w2q = np.empty((128, NS, KD, KQ, 128), ml_dtypes.bfloat16)
            b1q = np.empty((128, NS, KQ), np.float32)
            for jj in range(NS):
                e = slot_expert[g][jj]
                w1e = w1[e][:, q * FQ:(q + 1) * FQ]        # [D, FQ]
                w1q[:, jj] = w1e.reshape(KD, 128, KQ, 128).transpose(
                    1, 2, 0, 3).astype(ml_dtypes.bfloat16)
                w2e = w2[e][q * FQ:(q + 1) * FQ, :]        # [FQ, D]
                w2q[:, jj] = w2e.reshape(KQ, 128, KD, 128).transpose(
                    1, 2, 0, 3).astype(ml_dtypes.bfloat16)
                b1q[:, jj] = b1[e][q * FQ:(q + 1) * FQ].reshape(KQ, 128).T
            in_maps[g * NQ + q] = {"xT": xTg, "w1": w1q, "w2": w2q,
                                   "b1": b1q}

    res = bass_utils.run_bass_kernel_spmd(nc, in_maps,
                                          core_ids=list(range(N_CORES)),
                                          trace=TRACE)
    global LAST_RESULT
    LAST_RESULT = res

    out = np.empty((T, D), np.float32)
    for g in range(2):
        acc = res.results[g * NQ]["yT"].astype(np.float32)
        for q in range(1, NQ):
            acc += res.results[g * NQ + q]["yT"].astype(np.float32)
        # tile-packed [128, KD*CT] -> [D, CT] -> [CT, D]
        y3 = np.empty((KD, 128, CT), np.float32)
        fo = 0
        for (_, n0, w) in sched:
            y3[:, :, n0:n0 + w] = acc[:, fo:fo + KD * w].reshape(
                128, KD, w).transpose(1, 0, 2)
            fo += KD * w
        yg = y3.reshape(D, CT).T
        for e, idx, pos in tok_idx[g]:
            out[idx] = yg[pos:pos + len(idx)] + b2[e]
    return out.reshape(B, S, D)
